# revision 7
# baseline (speedup 1.0000x reference)
"""Two-layer GCN (BotGCN) on 8 Trainium2 NeuronCores.

Distribution: nodes partitioned contiguously across the 8 cores (12500
each). Each core owns the edges whose destination lands in its block.
Layer math is refactored so all per-edge work is a gather of pre-scaled
rows + a segment-sum:

    out[v] = dinv[v] * (sum_{e: dst=v, real} (dinv[src] * h[src])
                        + dinv[v] * h[v]) + bias

Self-loops are the elementwise term dinv[v]^2 * h[v], added from an
SBUF-resident copy of the local pre-scaled rows.

v2 structure (vs the first working version):
  - The gather table is PIECE-major: local nodes are split into 4 pieces
    (3200/3200/3200/2900 rows). Piece p of the table is
    [8 cores x PSZ[p] rows x 256B], produced by its own AllGather, so
    collectives stream piece-by-piece and overlap with compute instead
    of forming one big barrier per layer.
  - Destination blocks are processed in 4 passes of 25/25/25/23 blocks
    aligned with the pieces, so layer-1 post for pass p feeds AllGather
    piece p of layer 2 while later passes still compute.
  - One-hot segment-sum matrices are generated ON-CHIP per chunk with a
    single DVE tensor_tensor is_equal (iota row broadcast vs per-slot
    dstrel broadcast) instead of streaming a precomputed [128, S] bf16
    matrix from DRAM (saves ~35MB/layer/core of HBM traffic).
  - Gather windows == pieces (<=25616 rows, int16-safe indices).

Per chunk (40 groups of 128 slots): 1 DVE one-hot, 4 dma_gathers (one
per SWDGE queue), 40 PE matmuls accumulating into per-pass PSUM banks.
"""

import numpy as np

N = 100000
NCORES = 8
NPC = N // NCORES            # 12500 nodes per core
BLK = 128
NBLK = (NPC + BLK - 1) // BLK          # 98 destination blocks
LAST_BLK = NPC - (NBLK - 1) * BLK      # 84 nodes in the last block
F_IN, F_HID, F_OUT = 128, 64, 2
TBLW = 128                              # bf16 table row width (256B)
NP_ = 4                                 # pieces / passes
PASS_BLOCKS = [list(range(0, 25)), list(range(25, 50)),
               list(range(50, 75)), list(range(75, NBLK))]
PSTART = [0, 3200, 6400, 9600]          # local node offset of each piece
PSZ = [3200, 3200, 3200, 2900]          # local nodes per piece
PSZE = [q + 2 for q in PSZ]             # + 2 zero rows per core shard
CHUNK_GROUPS = 40                       # 5120 slots per chunk

_CACHE = {}

# Results of the most recent run (for the local test harness's profiling).
LAST_RESULTS = None


def _preprocess(edge_index):
    """Host-side integer bucketing of the edge list (self-loops excluded).

    Bucket key = (dst block, src piece); bucket sizes are the max over
    cores (shared NEFF), in units of 128 slots. Returns the group table
    G[b, w], slot count S, and per-core staged gather-index / dstrel
    arrays.
    """
    src = np.asarray(edge_index[0]).astype(np.int64)
    dst = np.asarray(edge_index[1]).astype(np.int64)

    # degree includes the self-loop (reference semantics)
    deg = (np.bincount(dst, minlength=N) + 1).astype(np.float32)

    core = dst // NPC
    dloc = dst % NPC
    blk = dloc // BLK
    scor = src // NPC
    sloc = src % NPC
    piece = np.minimum(sloc // 3200, NP_ - 1)

    cnt = np.zeros((NCORES, NBLK, NP_), np.int64)
    np.add.at(cnt, (core, blk, piece), 1)
    G = -(-cnt.max(axis=0) // BLK)        # [NBLK, NP_], shared by cores

    # Bucket ordering: (pass, window=piece, block) — must match the
    # kernel loops.
    bucket_order = []
    for p in range(NP_):
        for w in range(NP_):
            for b in PASS_BLOCKS[p]:
                bucket_order.append((b, w))
    nbuckets = len(bucket_order)
    ord_of = np.zeros((NBLK, NP_), np.int64)
    sizes = np.zeros(nbuckets, np.int64)
    wins = np.zeros(nbuckets, np.int64)
    for i, (b, w) in enumerate(bucket_order):
        ord_of[b, w] = i
        sizes[i] = G[b, w] * BLK
        wins[i] = w
    offs = np.zeros(nbuckets + 1, np.int64)
    np.cumsum(sizes, out=offs[1:])
    S = int(offs[-1])
    starts = offs[:-1]
    # window of every slot (for per-window zero-row pad indices)
    win_of_slot = np.repeat(wins, sizes)
    zrow = np.array([PSZ[w] for w in range(NP_)], np.int64)

    import ml_dtypes
    per_core = []
    for c in range(NCORES):
        m = core == c
        key = ord_of[blk[m], piece[m]]
        order = np.argsort(key, kind="stable")
        ks = key[order]
        bstart = np.searchsorted(ks, np.arange(nbuckets))
        rank = np.arange(len(ks)) - bstart[ks]
        slot = starts[ks] + rank

        so_cor = scor[m][order]
        so_loc = sloc[m][order]
        so_p = piece[m][order]
        row = so_cor * np.array(PSZE)[so_p] + (so_loc - np.array(PSTART)[so_p])

        gidx = zrow[win_of_slot].astype(np.int16)   # pad -> window zero row
        gidx[slot] = row.astype(np.int16)
        drel = np.full(S, -1.0, np.float32)
        drel[slot] = (dloc[m][order] % BLK).astype(np.float32)

        gidx16 = gidx.reshape(S // 16, 16).T      # [16, S/16]
        gidx_rep = np.tile(gidx16, (8, 1)).copy() # replicated for Q7 cores
        # dstrel per slot, [128, S/128]: partition = slot % 128, col = group
        drelG = np.ascontiguousarray(
            drel.reshape(S // BLK, BLK).T).astype(ml_dtypes.bfloat16)

        degc = np.ones(NBLK * BLK, np.float32)
        degc[:NPC] = deg[c * NPC:(c + 1) * NPC]
        degT = degc.reshape(NBLK, BLK).T.copy()   # [128, NBLK]

        per_core.append({"gidx": gidx_rep, "drelG": drelG, "degT": degT})

    return G, S, per_core


def _build(G, S, b1_nonzero, b2_nonzero):
    import concourse.bacc as bacc
    import concourse.mybir as mybir
    import concourse.tile as tile
    from concourse.masks import make_identity

    f32 = mybir.dt.float32
    bf16 = mybir.dt.bfloat16
    AT = mybir.AluOpType

    # first/last matmul (w, b, g) per (pass, bank) for start/stop flags.
    first, last = {}, {}
    for p in range(NP_):
        for w in range(NP_):
            for b in PASS_BLOCKS[p]:
                bank = PASS_BLOCKS[p].index(b) // 8
                for g in range(int(G[b, w])):
                    last[(p, bank)] = (w, b, g)
                    first.setdefault((p, bank), (w, b, g))

    nc = bacc.Bacc("TRN2", target_bir_lowering=False, debug=False,
                   enable_asserts=False, num_devices=NCORES,
                   num_swdge_queues=4)
    xT = nc.dram_tensor("xT", [F_IN, NPC], bf16, kind="ExternalInput")
    W1 = nc.dram_tensor("W1", [F_IN, F_HID], bf16, kind="ExternalInput")
    W2 = nc.dram_tensor("W2", [F_HID, F_OUT], f32, kind="ExternalInput")
    b1r = nc.dram_tensor("b1r", [BLK, F_HID], f32, kind="ExternalInput")
    b2r = nc.dram_tensor("b2r", [BLK, F_OUT], f32, kind="ExternalInput")
    degT = nc.dram_tensor("degT", [BLK, NBLK], f32, kind="ExternalInput")
    gidx = nc.dram_tensor("gidx", [BLK, S // 16], mybir.dt.int16,
                          kind="ExternalInput")
    drelG = nc.dram_tensor("drelG", [BLK, S // BLK], bf16,
                           kind="ExternalInput")
    iotab = nc.dram_tensor("iotab", [BLK, BLK], bf16, kind="ExternalInput")
    y = nc.dram_tensor("y", [NPC, F_OUT], f32, kind="ExternalOutput")

    with tile.TileContext(nc) as tc:
        with tc.tile_pool(name="const", bufs=1) as const, \
             tc.tile_pool(name="xt", bufs=3) as xpool, \
             tc.tile_pool(name="hs", bufs=3) as hpool, \
             tc.tile_pool(name="msgs", bufs=6) as mpool, \
             tc.tile_pool(name="oh", bufs=4) as ohpool, \
             tc.tile_pool(name="post", bufs=3) as ppool, \
             tc.tile_pool(name="psb", bufs=1, space="PSUM") as psb, \
             tc.tile_pool(name="pst", bufs=2, space="PSUM") as pst, \
             tc.tile_pool(name="dram", bufs=1, space="DRAM") as dram:

            # per-layer, per-piece AllGather in/out DRAM tiles. The out
            # tile has 16 extra zero rows at 8*PSZ[p] for pad slots.
            ag_in = [[dram.tile([PSZE[p], TBLW], bf16,
                                name=f"agin{L}_{p}", tag=f"agin{L}_{p}")
                      for p in range(NP_)] for L in range(2)]
            ag_out = [[dram.tile([8 * PSZE[p], TBLW], bf16,
                                 addr_space="Shared",
                                 name=f"agout{L}_{p}", tag=f"agout{L}_{p}")
                       for p in range(NP_)] for L in range(2)]

            # ---- constants ----
            ident = const.tile([BLK, BLK], f32)
            make_identity(nc, ident[:])
            W1t = const.tile([F_IN, F_HID], bf16)
            nc.sync.dma_start(W1t[:], W1[:])
            W2t = const.tile([F_HID, F_OUT], f32)
            nc.sync.dma_start(W2t[:], W2[:])
            if b1_nonzero:
                b1t = const.tile([BLK, F_HID], f32)
                nc.sync.dma_start(b1t[:], b1r[:])
            if b2_nonzero:
                b2t = const.tile([BLK, F_OUT], f32)
                nc.sync.dma_start(b2t[:], b2r[:])
            degt = const.tile([BLK, NBLK], f32)
            nc.sync.dma_start(degt[:], degT[:])
            rcp = const.tile([BLK, NBLK], f32)
            nc.vector.reciprocal(rcp[:], degt[:])
            dinv = const.tile([BLK, NBLK], f32)
            nc.scalar.sqrt(dinv[:], rcp[:])
            dinv2 = const.tile([BLK, NBLK], f32)
            nc.vector.tensor_mul(dinv2[:], dinv[:], dinv[:])
            idx_sb = const.tile([BLK, S // 16], mybir.dt.int16)
            nc.sync.dma_start(idx_sb[:], gidx[:])
            drel_sb = const.tile([BLK, S // BLK], bf16)
            nc.sync.dma_start(drel_sb[:], drelG[:])
            iota_t = const.tile([BLK, BLK], bf16)
            nc.sync.dma_start(iota_t[:], iotab[:])
            zt = const.tile([2, TBLW], bf16)
            nc.gpsimd.memset(zt[:], 0.0)
            for L in range(2):
                for p in range(NP_):
                    nc.sync.dma_start(
                        ag_in[L][p][PSZ[p]:PSZ[p] + 2, :], zt[:])

            # SBUF-resident fp32 copies of the local pre-scaled rows for
            # the elementwise self-loop term (dinv^2 * h == dinv * hs).
            hs1_all = const.tile([BLK, NBLK * F_HID], f32)
            hs2_all = const.tile([BLK, NBLK * F_HID], f32)
            nc.gpsimd.memset(hs1_all[:], 0.0)
            nc.gpsimd.memset(hs2_all[:], 0.0)

            def piece_of_block(b):
                return min(b // 25, NP_ - 1)

            # ---- phase 1: h_scaled = dinv * (x @ W1), locally owned ----
            for p in range(NP_):
                for t in PASS_BLOCKS[p]:
                    nt = BLK if t < NBLK - 1 else LAST_BLK
                    xt = xpool.tile([F_IN, BLK], bf16, tag="xt")
                    nc.sync.dma_start(xt[:, :nt], xT[:, t * BLK:t * BLK + nt])
                    hp = pst.tile([BLK, 512], f32, space="PSUM", tag="tmp",
                                  name="hp")
                    nc.tensor.matmul(out=hp[:nt, :F_HID], lhsT=xt[:, :nt],
                                     rhs=W1t[:], start=True, stop=True)
                    nc.vector.tensor_scalar(
                        out=hs1_all[:nt, t * F_HID:(t + 1) * F_HID],
                        in0=hp[:nt, :F_HID],
                        scalar1=dinv[:nt, t:t + 1], scalar2=None,
                        op0=AT.mult)
                    hsb = hpool.tile([BLK, TBLW], bf16, tag="hs")
                    nc.scalar.activation(
                        hsb[:nt, :F_HID],
                        hs1_all[:nt, t * F_HID:(t + 1) * F_HID],
                        func=mybir.ActivationFunctionType.Copy)
                    r0 = t * BLK - PSTART[p]
                    nc.sync.dma_start(ag_in[0][p][r0:r0 + nt, :], hsb[:nt, :])
                with tc.high_priority():
                    nc.gpsimd.collective_compute(
                        "AllGather", AT.bypass,
                        replica_groups=[list(range(NCORES))],
                        ins=[ag_in[0][p].opt()],
                        outs=[ag_out[0][p].opt()],
                    )

            def run_layer(L, post_fn, after_pass=None):
                tables = ag_out[L]
                gslot = 0
                for p in range(NP_):
                    blocks = PASS_BLOCKS[p]
                    pos = {b: divmod(i, 8) for i, b in enumerate(blocks)}
                    banks = {}
                    for b in blocks:
                        bank, _ = pos[b]
                        if bank not in banks:
                            banks[bank] = psb.tile([BLK, 512], f32,
                                                   space="PSUM",
                                                   name=f"bank{bank}",
                                                   tag=f"bank{bank}")
                    for w in range(NP_):
                        groups = [(b, g) for b in blocks
                                  for g in range(int(G[b, w]))]
                        ci = 0
                        while ci < len(groups):
                            chunk = groups[ci:ci + CHUNK_GROUPS]
                            ci += len(chunk)
                            ng = len(chunk)
                            mt = mpool.tile([BLK, CHUNK_GROUPS, TBLW], bf16,
                                            tag="msgs")
                            ohc = ohpool.tile([BLK, CHUNK_GROUPS, BLK], bf16,
                                              tag="oh")
                            # one-hot on-chip: ohc[s, j, c] = (drel[s, j]==c)
                            nc.vector.tensor_tensor(
                                out=ohc[:, :ng, :],
                                in0=iota_t[:, :].unsqueeze(1)
                                    .to_broadcast([BLK, ng, BLK]),
                                in1=drel_sb[:, gslot:gslot + ng]
                                    .to_broadcast([BLK, ng, BLK]),
                                op=AT.is_equal)
                            # split across the 4 SWDGE queues
                            nsub = min(4, ng)
                            base, rem = divmod(ng, nsub)
                            j0 = 0
                            for si in range(nsub):
                                sg = base + (1 if si < rem else 0)
                                if sg == 0:
                                    continue
                                sn = sg * BLK
                                soff = gslot + j0
                                nc.gpsimd.dma_gather(
                                    out_ap=mt[:, j0:j0 + sg, :],
                                    in_ap=tables[w][:, :],
                                    idxs_ap=idx_sb[:, soff * 8:
                                                   soff * 8 + sn // 16],
                                    num_idxs=sn, num_idxs_reg=sn,
                                    elem_size=TBLW,
                                    single_packet=False,
                                    queue_num=si,
                                )
                                j0 += sg
                            for j, (b, g) in enumerate(chunk):
                                bank, off = pos[b]
                                nc.tensor.matmul(
                                    out=banks[bank][:, off * F_HID:
                                                    (off + 1) * F_HID],
                                    lhsT=ohc[:, j, :],
                                    rhs=mt[:, j, :F_HID],
                                    start=((w, b, g) == first[(p, bank)]),
                                    stop=((w, b, g) == last[(p, bank)]),
                                    skip_group_check=True)
                                gslot += 1
                    # read each PSUM bank back, run per-block post on SBUF
                    for bank, bt in banks.items():
                        bank_blocks = [b for b in blocks
                                       if pos[b][0] == bank]
                        post_fn(bank, bt, bank_blocks)
                    if after_pass is not None:
                        after_pass(p)

            # ---- layer 1 post:
            # X = bank + dinv*hs1 (self-loop); h1s = dinv*relu(dinv*X + b1)
            # b1 == 0 fast path: dinv*relu(dinv*X) == dinv^2*relu(X).
            def post1(bank, bt, bank_blocks):
                for i, b in enumerate(bank_blocks):
                    nb = BLK if b < NBLK - 1 else LAST_BLK
                    sl = bt[:, i * F_HID:(i + 1) * F_HID]
                    hb = hs1_all[:, b * F_HID:(b + 1) * F_HID]
                    x = ppool.tile([BLK, F_HID], f32, tag="post1x", name="x")
                    nc.vector.tensor_add(out=x[:], in0=hb, in1=sl)
                    sl2 = hs2_all[:, b * F_HID:(b + 1) * F_HID]
                    if b1_nonzero:
                        h = ppool.tile([BLK, F_HID], f32, tag="post1",
                                       name="h")
                        nc.vector.tensor_scalar(out=h[:], in0=x[:],
                                                scalar1=dinv[:, b:b + 1],
                                                scalar2=None, op0=AT.mult)
                        nc.vector.tensor_add(out=h[:], in0=h[:], in1=b1t[:])
                        nc.vector.tensor_scalar(out=sl2, in0=h[:],
                                                scalar1=dinv[:, b:b + 1],
                                                scalar2=0.0, op0=AT.mult,
                                                op1=AT.max)
                    else:
                        nc.scalar.activation(
                            x[:], x[:],
                            func=mybir.ActivationFunctionType.Relu)
                        nc.vector.tensor_scalar(out=sl2, in0=x[:],
                                                scalar1=dinv2[:, b:b + 1],
                                                scalar2=None, op0=AT.mult)
                    hbf = ppool.tile([BLK, TBLW], bf16, tag="post1b",
                                     name="hbf")
                    nc.scalar.activation(
                        hbf[:, :F_HID], sl2,
                        func=mybir.ActivationFunctionType.Copy)
                    pp = piece_of_block(b)
                    r0 = b * BLK - PSTART[pp]
                    nc.sync.dma_start(ag_in[1][pp][r0:r0 + nb, :],
                                      hbf[:nb, :])

            def ag2_piece(p):
                with tc.high_priority():
                    nc.gpsimd.collective_compute(
                        "AllGather", AT.bypass,
                        replica_groups=[list(range(NCORES))],
                        ins=[ag_in[1][p].opt()],
                        outs=[ag_out[1][p].opt()],
                    )

            run_layer(0, post1, after_pass=ag2_piece)

            # ---- layer 2 post: out = dinv*((bank + dinv*hs2) @ W2) + b2 --
            def post2(bank, bt, bank_blocks):
                for i, b in enumerate(bank_blocks):
                    nb = BLK if b < NBLK - 1 else LAST_BLK
                    sl = bt[:, i * F_HID:(i + 1) * F_HID]
                    hb = hs2_all[:, b * F_HID:(b + 1) * F_HID]
                    ag = ppool.tile([BLK, F_HID], f32, tag="agg2", name="ag")
                    nc.vector.tensor_add(out=ag[:], in0=hb, in1=sl)
                    t2 = pst.tile([BLK, 512], f32, space="PSUM", tag="tmp",
                                  name="t2")
                    nc.tensor.transpose(
                        out=t2[0:F_HID, 0:BLK],
                        in_=ag[:],
                        identity=ident[:])
                    aT = ppool.tile([F_HID, BLK], f32, tag="aggT", name="aT")
                    nc.scalar.activation(aT[:], t2[0:F_HID, 0:BLK],
                                         func=mybir.ActivationFunctionType.Copy)
                    nc.tensor.matmul(out=t2[:, BLK:BLK + F_OUT], lhsT=aT[:],
                                     rhs=W2t[:], start=True, stop=True)
                    o = ppool.tile([BLK, F_OUT], f32, tag="out2", name="o")
                    nc.vector.tensor_scalar(out=o[:],
                                            in0=t2[:, BLK:BLK + F_OUT],
                                            scalar1=dinv[:, b:b + 1],
                                            scalar2=None, op0=AT.mult)
                    if b2_nonzero:
                        nc.vector.tensor_add(out=o[:], in0=o[:], in1=b2t[:])
                    nc.sync.dma_start(y[b * BLK:b * BLK + nb, :], o[:nb, :])

            run_layer(1, post2)

    nc.compile()
    return nc


def _to_bf16(a):
    import ml_dtypes
    return np.asarray(a, dtype=np.float32).astype(ml_dtypes.bfloat16)


def kernel(x, W1, b1, W2, b2, edge_index):
    global LAST_RESULTS
    from concourse.bass_utils import run_bass_kernel_spmd
    import ml_dtypes

    x = np.asarray(x, dtype=np.float32)
    W1 = np.asarray(W1, dtype=np.float32)
    W2 = np.asarray(W2, dtype=np.float32)
    b1 = np.asarray(b1, dtype=np.float32)
    b2 = np.asarray(b2, dtype=np.float32)

    ekey = hash(np.asarray(edge_index).tobytes()) ^ hash(
        (bool(np.any(b1)), bool(np.any(b2))))
    if ekey in _CACHE:
        nc, G, S, per_core = _CACHE[ekey]
    else:
        G, S, per_core = _preprocess(edge_index)
        nc = _build(G, S, bool(np.any(b1)), bool(np.any(b2)))
        _CACHE.clear()
        _CACHE[ekey] = (nc, G, S, per_core)

    b1r = np.broadcast_to(b1, (BLK, F_HID)).copy()
    b2r = np.broadcast_to(b2, (BLK, F_OUT)).copy()
    W1b = _to_bf16(W1)
    iotab = np.tile(np.arange(BLK, dtype=np.float32), (BLK, 1)).astype(
        ml_dtypes.bfloat16)
    in_maps = []
    for c in range(NCORES):
        pc = per_core[c]
        in_maps.append({
            "xT": _to_bf16(np.ascontiguousarray(x[c * NPC:(c + 1) * NPC].T)),
            "W1": W1b, "W2": W2, "b1r": b1r, "b2r": b2r,
            "degT": pc["degT"], "gidx": pc["gidx"], "drelG": pc["drelG"],
            "iotab": iotab,
        })

    res = run_bass_kernel_spmd(nc, in_maps, core_ids=list(range(NCORES)))
    LAST_RESULTS = res
    return np.concatenate([res.results[c]["y"] for c in range(NCORES)], axis=0)


# revision 11
# speedup vs baseline: 1.1424x; 1.1424x over previous
"""Two-layer GCN (BotGCN) on 8 Trainium2 NeuronCores.

Distribution: nodes partitioned contiguously across the 8 cores (12500
each). Each core owns the edges whose destination lands in its block.
Layer math is refactored so all per-edge work is a gather of pre-scaled
rows + a segment-sum:

    out[v] = dinv[v] * (sum_{e: dst=v, real} (dinv[src] * h[src])
                        + dinv[v] * h[v]) + bias

Self-loops are the elementwise term dinv[v]^2 * h[v], added from an
SBUF-resident copy of the local pre-scaled rows.

Structure:
  - Gather table is PIECE-major: local nodes split into 4 pieces
    (3200/3200/3200/2900 + 2 zero rows per core); piece p of the table
    is [8 x PSZE[p] x 256B], produced by its own AllGather so the
    collectives stream piece-by-piece and overlap with compute.
  - Destinations processed in 4 passes of 25/25/25/23 blocks aligned
    with the pieces; layer-1 post for pass p feeds AllGather piece p of
    layer 2 while later passes still compute. Explicit dependency edges
    pin each AG trigger early in the Pool stream (the collective must
    live on the Pool engine) so its ~40us mesh hides under compute.
  - Slots are SEGMENT-packed: per (block, piece) segment rounded to 16
    slots (max over cores), segments concatenated per (pass, piece)
    section and the section padded to 128. 128-slot groups may span two
    blocks; each (group, block) pair gets its own matmul whose one-hot
    is generated on-chip: is_equal(code[slot], bip*128 + c) with
    code = block-in-pass*128 + dst%128 (f32 exact), against an f32
    iota-base table. ~11% padding vs ~21% for per-(block,piece)
    rounding to 128.
  - One DVE is_equal per matmul, 4 dma_gathers per chunk (one per SWDGE
    queue), PE matmuls accumulate into per-pass PSUM banks.
"""

import numpy as np

N = 100000
NCORES = 8
NPC = N // NCORES            # 12500 nodes per core
BLK = 128
NBLK = (NPC + BLK - 1) // BLK          # 98 destination blocks
LAST_BLK = NPC - (NBLK - 1) * BLK      # 84 nodes in the last block
F_IN, F_HID, F_OUT = 128, 64, 2
TBLW = 128                              # bf16 table row width (256B)
NP_ = 4                                 # pieces / passes
PASS_BLOCKS = [list(range(0, 25)), list(range(25, 50)),
               list(range(50, 75)), list(range(75, NBLK))]
PSTART = [0, 3200, 6400, 9600]          # local node offset of each piece
PSZ = [3200, 3200, 3200, 2900]          # local nodes per piece
PSZE = [q + 2 for q in PSZ]             # + 2 zero rows per core shard
CHUNK_GROUPS = 40                       # max 128-slot groups per chunk
MAXBIP = 25                             # max blocks per pass

_CACHE = {}

# Results of the most recent run (for the local test harness's profiling).
LAST_RESULTS = None


def _preprocess(edge_index):
    """Host-side integer bucketing of the edge list (self-loops excluded).

    Per (block, piece) segment sized to the max over cores, rounded to
    16 slots; segments concatenated per (pass, piece) section, section
    padded to a 128 multiple. Returns the section descriptors, slot
    count S, and per-core staged gather-index / code arrays.
    """
    src = np.asarray(edge_index[0]).astype(np.int64)
    dst = np.asarray(edge_index[1]).astype(np.int64)

    # degree includes the self-loop (reference semantics)
    deg = (np.bincount(dst, minlength=N) + 1).astype(np.float32)

    core = dst // NPC
    dloc = dst % NPC
    blk = dloc // BLK
    scor = src // NPC
    sloc = src % NPC
    piece = np.minimum(sloc // 3200, NP_ - 1)

    cnt = np.zeros((NCORES, NBLK, NP_), np.int64)
    np.add.at(cnt, (core, blk, piece), 1)
    seg = (-(-cnt.max(axis=0) // 16) * 16).astype(np.int64)   # [98, 4]

    seg_id_of = np.zeros((NBLK, NP_), np.int64)
    seg_sizes = []
    seg_win = []
    sections = []
    sid = 0
    slot_base = 0
    for p in range(NP_):
        for w in range(NP_):
            sec_start = slot_base
            bounds = []
            off = 0
            for b in PASS_BLOCKS[p]:
                seg_id_of[b, w] = sid
                seg_sizes.append(int(seg[b, w]))
                seg_win.append(w)
                bounds.append((b, off, off + int(seg[b, w])))
                off += int(seg[b, w])
                sid += 1
            tot128 = -(-off // 128) * 128
            if tot128 > off:
                seg_sizes.append(tot128 - off)   # section pad pseudo-segment
                seg_win.append(w)
                sid += 1
            slot_base += tot128
            ngroups = tot128 // 128
            mms = []
            for j in range(ngroups):
                lo, hi = j * 128, (j + 1) * 128
                for (b, s0, s1) in bounds:
                    if s0 < hi and s1 > lo:
                        mms.append((j, b))
            sections.append({"pass": p, "w": w, "ngroups": ngroups,
                             "mms": mms, "start_slot": sec_start})
    S = slot_base
    nseg = sid
    seg_sizes = np.array(seg_sizes, np.int64)
    seg_win = np.array(seg_win, np.int64)
    seg_offs = np.zeros(nseg + 1, np.int64)
    np.cumsum(seg_sizes, out=seg_offs[1:])
    win_of_slot = np.repeat(seg_win, seg_sizes)
    zrow = np.array([PSZ[w] for w in range(NP_)], np.int64)

    bip = np.zeros(NBLK, np.int64)
    for p in range(NP_):
        for i, b in enumerate(PASS_BLOCKS[p]):
            bip[b] = i

    per_core = []
    for c in range(NCORES):
        m = core == c
        key = seg_id_of[blk[m], piece[m]]
        order = np.argsort(key, kind="stable")
        ks = key[order]
        bstart = np.searchsorted(ks, np.arange(nseg))
        rank = np.arange(len(ks)) - bstart[ks]
        slot = seg_offs[ks] + rank

        so_cor = scor[m][order]
        so_loc = sloc[m][order]
        so_p = piece[m][order]
        row = so_cor * np.array(PSZE)[so_p] + (so_loc - np.array(PSTART)[so_p])

        gidx = zrow[win_of_slot].astype(np.int16)   # pad -> window zero row
        gidx[slot] = row.astype(np.int16)
        code = np.full(S, -1.0, np.float32)
        code[slot] = (bip[blk[m][order]] * BLK
                      + dloc[m][order] % BLK).astype(np.float32)

        gidx16 = gidx.reshape(S // 16, 16).T      # [16, S/16]
        gidx_rep = np.tile(gidx16, (8, 1)).copy() # replicated for Q7 cores
        # code per slot, [128, S/128]: partition = slot % 128, col = group
        codeG = np.ascontiguousarray(code.reshape(S // BLK, BLK).T)

        degc = np.ones(NBLK * BLK, np.float32)
        degc[:NPC] = deg[c * NPC:(c + 1) * NPC]
        degT = degc.reshape(NBLK, BLK).T.copy()   # [128, NBLK]

        per_core.append({"gidx": gidx_rep, "codeG": codeG, "degT": degT})

    return sections, S, per_core


def _chunks_of(n):
    """Split n groups into near-equal chunks of at most CHUNK_GROUPS."""
    k = -(-n // CHUNK_GROUPS)
    base, rem = divmod(n, k)
    out = []
    s = 0
    for i in range(k):
        sz = base + (1 if i < rem else 0)
        out.append((s, sz))
        s += sz
    return out


def _build(sections, S, b1_nonzero, b2_nonzero):
    import concourse.bacc as bacc
    import concourse.mybir as mybir
    import concourse.tile as tile
    from concourse.masks import make_identity
    from bass_rust import add_dep_helper

    f32 = mybir.dt.float32
    bf16 = mybir.dt.bfloat16
    AT = mybir.AluOpType

    bip = {}
    pos = {}
    for p in range(NP_):
        for i, b in enumerate(PASS_BLOCKS[p]):
            bip[b] = i
            pos[b] = divmod(i, 8)

    # first/last matmul (global emission index) per (pass, bank)
    first, last = {}, {}
    mmidx = 0
    for sec in sections:
        p = sec["pass"]
        for (j, b) in sec["mms"]:
            bank, _ = pos[b]
            last[(p, bank)] = mmidx
            first.setdefault((p, bank), mmidx)
            mmidx += 1

    nc = bacc.Bacc("TRN2", target_bir_lowering=False, debug=False,
                   enable_asserts=False, num_devices=NCORES,
                   num_swdge_queues=4)
    xT = nc.dram_tensor("xT", [F_IN, NPC], bf16, kind="ExternalInput")
    W1 = nc.dram_tensor("W1", [F_IN, F_HID], bf16, kind="ExternalInput")
    W2 = nc.dram_tensor("W2", [F_HID, F_OUT], f32, kind="ExternalInput")
    b1r = nc.dram_tensor("b1r", [BLK, F_HID], f32, kind="ExternalInput")
    b2r = nc.dram_tensor("b2r", [BLK, F_OUT], f32, kind="ExternalInput")
    degT = nc.dram_tensor("degT", [BLK, NBLK], f32, kind="ExternalInput")
    gidx = nc.dram_tensor("gidx", [BLK, S // 16], mybir.dt.int16,
                          kind="ExternalInput")
    codeG = nc.dram_tensor("codeG", [BLK, S // BLK], f32,
                           kind="ExternalInput")
    iotab = nc.dram_tensor("iotab", [BLK, MAXBIP * BLK], f32,
                           kind="ExternalInput")
    y = nc.dram_tensor("y", [NPC, F_OUT], f32, kind="ExternalOutput")

    with tile.TileContext(nc) as tc:
        with tc.tile_pool(name="const", bufs=1) as const, \
             tc.tile_pool(name="xt", bufs=3) as xpool, \
             tc.tile_pool(name="hs", bufs=3) as hpool, \
             tc.tile_pool(name="msgs", bufs=6) as mpool, \
             tc.tile_pool(name="oh", bufs=3) as ohpool, \
             tc.tile_pool(name="post", bufs=3) as ppool, \
             tc.tile_pool(name="psb", bufs=1, space="PSUM") as psb, \
             tc.tile_pool(name="pst", bufs=2, space="PSUM") as pst, \
             tc.tile_pool(name="dram", bufs=1, space="DRAM") as dram:

            ag_in = [[dram.tile([PSZE[p], TBLW], bf16,
                                name=f"agin{L}_{p}", tag=f"agin{L}_{p}")
                      for p in range(NP_)] for L in range(2)]
            ag_out = [[dram.tile([8 * PSZE[p], TBLW], bf16,
                                 addr_space="Shared",
                                 name=f"agout{L}_{p}", tag=f"agout{L}_{p}")
                       for p in range(NP_)] for L in range(2)]

            # ---- constants ----
            ident = const.tile([BLK, BLK], f32)
            make_identity(nc, ident[:])
            W1t = const.tile([F_IN, F_HID], bf16)
            nc.sync.dma_start(W1t[:], W1[:])
            W2t = const.tile([F_HID, F_OUT], f32)
            nc.sync.dma_start(W2t[:], W2[:])
            if b1_nonzero:
                b1t = const.tile([BLK, F_HID], f32)
                nc.sync.dma_start(b1t[:], b1r[:])
            if b2_nonzero:
                b2t = const.tile([BLK, F_OUT], f32)
                nc.sync.dma_start(b2t[:], b2r[:])
            degt = const.tile([BLK, NBLK], f32)
            nc.sync.dma_start(degt[:], degT[:])
            rcp = const.tile([BLK, NBLK], f32)
            nc.vector.reciprocal(rcp[:], degt[:])
            dinv = const.tile([BLK, NBLK], f32)
            nc.scalar.sqrt(dinv[:], rcp[:])
            dinv2 = const.tile([BLK, NBLK], f32)
            nc.vector.tensor_mul(dinv2[:], dinv[:], dinv[:])
            idx_sb = const.tile([BLK, S // 16], mybir.dt.int16)
            nc.sync.dma_start(idx_sb[:], gidx[:])
            code_sb = const.tile([BLK, S // BLK], f32)
            nc.sync.dma_start(code_sb[:], codeG[:])
            iota_sb = const.tile([BLK, MAXBIP * BLK], f32)
            nc.sync.dma_start(iota_sb[:], iotab[:])
            zt = const.tile([2, TBLW], bf16)
            nc.gpsimd.memset(zt[:], 0.0)
            for L in range(2):
                for p in range(NP_):
                    nc.sync.dma_start(
                        ag_in[L][p][PSZ[p]:PSZ[p] + 2, :], zt[:])

            # SBUF-resident fp32 copies of the local pre-scaled rows for
            # the elementwise self-loop term (dinv^2 * h == dinv * hs).
            hs1_all = const.tile([BLK, NBLK * F_HID], f32)
            hs2_all = const.tile([BLK, NBLK * F_HID], f32)
            nc.gpsimd.memset(hs1_all[:], 0.0)
            nc.gpsimd.memset(hs2_all[:], 0.0)

            def piece_of_block(b):
                return min(b // 25, NP_ - 1)

            # ---- phase 1: h_scaled = dinv * (x @ W1), locally owned ----
            for p in range(NP_):
                for t in PASS_BLOCKS[p]:
                    nt = BLK if t < NBLK - 1 else LAST_BLK
                    xt = xpool.tile([F_IN, BLK], bf16, tag="xt")
                    nc.sync.dma_start(xt[:, :nt], xT[:, t * BLK:t * BLK + nt])
                    hp = pst.tile([BLK, 512], f32, space="PSUM", tag="tmp",
                                  name="hp")
                    nc.tensor.matmul(out=hp[:nt, :F_HID], lhsT=xt[:, :nt],
                                     rhs=W1t[:], start=True, stop=True)
                    nc.vector.tensor_scalar(
                        out=hs1_all[:nt, t * F_HID:(t + 1) * F_HID],
                        in0=hp[:nt, :F_HID],
                        scalar1=dinv[:nt, t:t + 1], scalar2=None,
                        op0=AT.mult)
                    hsb = hpool.tile([BLK, TBLW], bf16, tag="hs")
                    nc.scalar.activation(
                        hsb[:nt, :F_HID],
                        hs1_all[:nt, t * F_HID:(t + 1) * F_HID],
                        func=mybir.ActivationFunctionType.Copy)
                    r0 = t * BLK - PSTART[p]
                    nc.sync.dma_start(ag_in[0][p][r0:r0 + nt, :], hsb[:nt, :])
                nc.gpsimd.collective_compute(
                    "AllGather", AT.bypass,
                    replica_groups=[list(range(NCORES))],
                    ins=[ag_in[0][p].opt()],
                    outs=[ag_out[0][p].opt()],
                )

            ag2_insts = {}
            anchor = {}       # (L, pass, w, chunk) -> first gather inst

            def run_layer(L, post_fn, after_pass=None):
                tables = ag_out[L]
                mmcount = [0]
                banks = {}

                for sec in sections:
                    p, w = sec["pass"], sec["w"]
                    blocks = PASS_BLOCKS[p]
                    if w == 0:
                        banks.clear()
                        for b in blocks:
                            bank, _ = pos[b]
                            if bank not in banks:
                                banks[bank] = psb.tile(
                                    [BLK, 512], f32, space="PSUM",
                                    name=f"bank{bank}", tag=f"bank{bank}")
                    g0_global = sec["start_slot"] // BLK
                    chunks = _chunks_of(sec["ngroups"])
                    for ci, (cs, cn) in enumerate(chunks):
                        mt = mpool.tile([BLK, CHUNK_GROUPS, TBLW], bf16,
                                        tag="msgs")
                        nsub = min(4, cn)
                        base, rem = divmod(cn, nsub)
                        j0 = 0
                        for si in range(nsub):
                            sg = base + (1 if si < rem else 0)
                            if sg == 0:
                                continue
                            sn = sg * BLK
                            soff = g0_global + cs + j0
                            gi = nc.gpsimd.dma_gather(
                                out_ap=mt[:, j0:j0 + sg, :],
                                in_ap=tables[w][:, :],
                                idxs_ap=idx_sb[:, soff * 8:
                                               soff * 8 + sn // 16],
                                num_idxs=sn, num_idxs_reg=sn,
                                elem_size=TBLW,
                                single_packet=False,
                                queue_num=si,
                            )
                            if si == 0:
                                anchor[(L, p, w, ci)] = gi
                            j0 += sg
                        mlist = [(j, b) for (j, b) in sec["mms"]
                                 if cs <= j < cs + cn]
                        assert len(mlist) <= 56, len(mlist)
                        ohc = ohpool.tile([BLK, 56, BLK], bf16, tag="oh")
                        for mi, (j, b) in enumerate(mlist):
                            bi = bip[b]
                            nc.vector.tensor_tensor(
                                out=ohc[:, mi, :],
                                in0=iota_sb[:, bi * BLK:(bi + 1) * BLK],
                                in1=code_sb[:, g0_global + j:
                                            g0_global + j + 1]
                                    .to_broadcast([BLK, BLK]),
                                op=AT.is_equal)
                        for mi, (j, b) in enumerate(mlist):
                            bank, off = pos[b]
                            gm = mmcount[0]
                            mmcount[0] += 1
                            nc.tensor.matmul(
                                out=banks[bank][:, off * F_HID:
                                                (off + 1) * F_HID],
                                lhsT=ohc[:, mi, :],
                                rhs=mt[:, j - cs, :F_HID],
                                start=(gm == first[(p, bank)]),
                                stop=(gm == last[(p, bank)]),
                                skip_group_check=True)
                    if w == NP_ - 1:
                        for bank, bt in banks.items():
                            bank_blocks = [b for b in blocks
                                           if pos[b][0] == bank]
                            post_fn(bank, bt, bank_blocks)
                        if after_pass is not None:
                            after_pass(p)

            # ---- layer 1 post:
            # X = bank + dinv*hs1 (self-loop); h1s = dinv*relu(dinv*X + b1)
            # b1 == 0 fast path: dinv*relu(dinv*X) == dinv^2*relu(X).
            def post1(bank, bt, bank_blocks):
                for i, b in enumerate(bank_blocks):
                    nb = BLK if b < NBLK - 1 else LAST_BLK
                    sl = bt[:, i * F_HID:(i + 1) * F_HID]
                    hb = hs1_all[:, b * F_HID:(b + 1) * F_HID]
                    x = ppool.tile([BLK, F_HID], f32, tag="post1x", name="x")
                    nc.vector.tensor_add(out=x[:], in0=hb, in1=sl)
                    sl2 = hs2_all[:, b * F_HID:(b + 1) * F_HID]
                    if b1_nonzero:
                        h = ppool.tile([BLK, F_HID], f32, tag="post1",
                                       name="h")
                        nc.vector.tensor_scalar(out=h[:], in0=x[:],
                                                scalar1=dinv[:, b:b + 1],
                                                scalar2=None, op0=AT.mult)
                        nc.vector.tensor_add(out=h[:], in0=h[:], in1=b1t[:])
                        nc.vector.tensor_scalar(out=sl2, in0=h[:],
                                                scalar1=dinv[:, b:b + 1],
                                                scalar2=0.0, op0=AT.mult,
                                                op1=AT.max)
                    else:
                        nc.scalar.activation(
                            x[:], x[:],
                            func=mybir.ActivationFunctionType.Relu)
                        nc.vector.tensor_scalar(out=sl2, in0=x[:],
                                                scalar1=dinv2[:, b:b + 1],
                                                scalar2=None, op0=AT.mult)
                    hbf = ppool.tile([BLK, TBLW], bf16, tag="post1b",
                                     name="hbf")
                    nc.scalar.activation(
                        hbf[:, :F_HID], sl2,
                        func=mybir.ActivationFunctionType.Copy)
                    pp = piece_of_block(b)
                    r0 = b * BLK - PSTART[pp]
                    nc.sync.dma_start(ag_in[1][pp][r0:r0 + nb, :],
                                      hbf[:nb, :])

            def ag2_piece(p):
                cc = nc.gpsimd.collective_compute(
                    "AllGather", AT.bypass,
                    replica_groups=[list(range(NCORES))],
                    ins=[ag_in[1][p].opt()],
                    outs=[ag_out[1][p].opt()],
                )
                ag2_insts[p] = cc

            run_layer(0, post1, after_pass=ag2_piece)

            # ---- layer 2 post: out = dinv*((bank + dinv*hs2) @ W2) + b2 --
            def post2(bank, bt, bank_blocks):
                for i, b in enumerate(bank_blocks):
                    nb = BLK if b < NBLK - 1 else LAST_BLK
                    sl = bt[:, i * F_HID:(i + 1) * F_HID]
                    hb = hs2_all[:, b * F_HID:(b + 1) * F_HID]
                    ag = ppool.tile([BLK, F_HID], f32, tag="agg2", name="ag")
                    nc.vector.tensor_add(out=ag[:], in0=hb, in1=sl)
                    t2 = pst.tile([BLK, 512], f32, space="PSUM", tag="tmp",
                                  name="t2")
                    nc.tensor.transpose(
                        out=t2[0:F_HID, 0:BLK],
                        in_=ag[:],
                        identity=ident[:])
                    aT = ppool.tile([F_HID, BLK], f32, tag="aggT", name="aT")
                    nc.scalar.activation(aT[:], t2[0:F_HID, 0:BLK],
                                         func=mybir.ActivationFunctionType.Copy)
                    nc.tensor.matmul(out=t2[:, BLK:BLK + F_OUT], lhsT=aT[:],
                                     rhs=W2t[:], start=True, stop=True)
                    o = ppool.tile([BLK, F_OUT], f32, tag="out2", name="o")
                    nc.vector.tensor_scalar(out=o[:],
                                            in0=t2[:, BLK:BLK + F_OUT],
                                            scalar1=dinv[:, b:b + 1],
                                            scalar2=None, op0=AT.mult)
                    if b2_nonzero:
                        nc.vector.tensor_add(out=o[:], in0=o[:], in1=b2t[:])
                    nc.sync.dma_start(y[b * BLK:b * BLK + nb, :], o[:nb, :])

            run_layer(1, post2)

            # Pin each AG2 trigger early in the Pool stream: a gather a
            # couple chunks into the NEXT pass (or layer-2 start for the
            # last pass) must come after it, so the scheduler cannot
            # defer the trigger to layer-2's first use of the table.
            for p in range(NP_):
                if p < NP_ - 1:
                    frm = anchor.get((0, p + 1, 0, 1)) or \
                          anchor.get((0, p + 1, 0, 0))
                else:
                    frm = anchor.get((1, 0, 0, 1)) or anchor.get((1, 0, 0, 0))
                if frm is not None and p in ag2_insts:
                    add_dep_helper(frm.ins, ag2_insts[p].ins, True,
                                   f"pin AG2_{p} trigger early")

    nc.compile()
    return nc


def _to_bf16(a):
    import ml_dtypes
    return np.asarray(a, dtype=np.float32).astype(ml_dtypes.bfloat16)


def kernel(x, W1, b1, W2, b2, edge_index):
    global LAST_RESULTS
    from concourse.bass_utils import run_bass_kernel_spmd

    x = np.asarray(x, dtype=np.float32)
    W1 = np.asarray(W1, dtype=np.float32)
    W2 = np.asarray(W2, dtype=np.float32)
    b1 = np.asarray(b1, dtype=np.float32)
    b2 = np.asarray(b2, dtype=np.float32)

    ekey = hash(np.asarray(edge_index).tobytes()) ^ hash(
        (bool(np.any(b1)), bool(np.any(b2))))
    if ekey in _CACHE:
        nc, sections, S, per_core = _CACHE[ekey]
    else:
        sections, S, per_core = _preprocess(edge_index)
        nc = _build(sections, S, bool(np.any(b1)), bool(np.any(b2)))
        _CACHE.clear()
        _CACHE[ekey] = (nc, sections, S, per_core)

    b1r = np.broadcast_to(b1, (BLK, F_HID)).copy()
    b2r = np.broadcast_to(b2, (BLK, F_OUT)).copy()
    W1b = _to_bf16(W1)
    # iota base table: col (bi*128 + c) has value bi*128 + c
    iotab = np.tile(np.arange(MAXBIP * BLK, dtype=np.float32), (BLK, 1))
    in_maps = []
    for c in range(NCORES):
        pc = per_core[c]
        in_maps.append({
            "xT": _to_bf16(np.ascontiguousarray(x[c * NPC:(c + 1) * NPC].T)),
            "W1": W1b, "W2": W2, "b1r": b1r, "b2r": b2r,
            "degT": pc["degT"], "gidx": pc["gidx"], "codeG": pc["codeG"],
            "iotab": iotab,
        })

    res = run_bass_kernel_spmd(nc, in_maps, core_ids=list(range(NCORES)))
    LAST_RESULTS = res
    return np.concatenate([res.results[c]["y"] for c in range(NCORES)], axis=0)


# revision 13
# speedup vs baseline: 1.2717x; 1.1132x over previous
"""Two-layer GCN (BotGCN) on 8 Trainium2 NeuronCores.

Distribution: nodes partitioned contiguously across the 8 cores (12500
each). Each core owns the edges whose destination lands in its block.
Layer math is refactored so all per-edge work is a gather of pre-scaled
rows + a segment-sum:

    out[v] = dinv[v] * (sum_{e: dst=v, real} (dinv[src] * h[src])
                        + dinv[v] * h[v]) + bias

Self-loops are the elementwise term dinv[v]^2 * h[v], added from an
SBUF-resident copy of the local pre-scaled rows.

Structure:
  - Gather table is PIECE-major: local nodes split into 4 pieces
    (3200/3200/3200/2900 + 2 zero rows per core); piece p of the table
    is [8 x PSZE[p] x 256B], produced by its own AllGather so the
    collectives stream piece-by-piece and overlap with compute.
  - Destinations processed in 4 passes of 25/25/25/23 blocks aligned
    with the pieces; layer-1 post for pass p feeds AllGather piece p of
    layer 2 while later passes still compute. Explicit dependency edges
    pin each AG trigger early in the Pool stream (the collective must
    live on the Pool engine) so its ~40us mesh hides under compute.
  - Slots are SEGMENT-packed: per (block, piece) segment rounded to 16
    slots (max over cores), segments concatenated per (pass, piece)
    section and the section padded to 128. 128-slot groups may span two
    blocks; each (group, block) pair gets its own matmul whose one-hot
    is generated on-chip: is_equal(code[slot], bip*128 + c) with
    code = block-in-pass*128 + dst%128 (f32 exact), against an f32
    iota-base table. ~11% padding vs ~21% for per-(block,piece)
    rounding to 128.
  - One DVE is_equal per matmul, 4 dma_gathers per chunk (one per SWDGE
    queue), PE matmuls accumulate into per-pass PSUM banks.
"""

import numpy as np

N = 100000
NCORES = 8
NPC = N // NCORES            # 12500 nodes per core
BLK = 128
NBLK = (NPC + BLK - 1) // BLK          # 98 destination blocks
LAST_BLK = NPC - (NBLK - 1) * BLK      # 84 nodes in the last block
F_IN, F_HID, F_OUT = 128, 64, 2
TBLW = 128                              # bf16 table row width (256B)
NP_ = 4                                 # pieces / passes
PASS_BLOCKS = [list(range(0, 25)), list(range(25, 50)),
               list(range(50, 75)), list(range(75, NBLK))]
PSTART = [0, 3200, 6400, 9600]          # local node offset of each piece
PSZ = [3200, 3200, 3200, 2900]          # local nodes per piece
PSZE = [q + 2 for q in PSZ]             # + 2 zero rows per core shard
CHUNK_GROUPS = 40                       # max 128-slot groups per chunk
MAXBIP = 25                             # max blocks per pass

_CACHE = {}

# Results of the most recent run (for the local test harness's profiling).
LAST_RESULTS = None


def _preprocess(edge_index):
    """Host-side integer bucketing of the edge list (self-loops excluded).

    Per (block, piece) segment sized to the max over cores, rounded to
    16 slots; segments concatenated per (pass, piece) section, section
    padded to a 128 multiple. Returns the section descriptors, slot
    count S, and per-core staged gather-index / code arrays.
    """
    src = np.asarray(edge_index[0]).astype(np.int64)
    dst = np.asarray(edge_index[1]).astype(np.int64)

    # degree includes the self-loop (reference semantics)
    deg = (np.bincount(dst, minlength=N) + 1).astype(np.float32)

    core = dst // NPC
    dloc = dst % NPC
    blk = dloc // BLK
    scor = src // NPC
    sloc = src % NPC
    piece = np.minimum(sloc // 3200, NP_ - 1)

    cnt = np.zeros((NCORES, NBLK, NP_), np.int64)
    np.add.at(cnt, (core, blk, piece), 1)
    seg = (-(-cnt.max(axis=0) // 16) * 16).astype(np.int64)   # [98, 4]

    seg_id_of = np.zeros((NBLK, NP_), np.int64)
    seg_sizes = []
    seg_win = []
    sections = []
    sid = 0
    slot_base = 0
    for p in range(NP_):
        for w in range(NP_):
            sec_start = slot_base
            bounds = []
            off = 0
            for b in PASS_BLOCKS[p]:
                seg_id_of[b, w] = sid
                seg_sizes.append(int(seg[b, w]))
                seg_win.append(w)
                bounds.append((b, off, off + int(seg[b, w])))
                off += int(seg[b, w])
                sid += 1
            tot128 = -(-off // 128) * 128
            if tot128 > off:
                seg_sizes.append(tot128 - off)   # section pad pseudo-segment
                seg_win.append(w)
                sid += 1
            slot_base += tot128
            ngroups = tot128 // 128
            mms = []
            for j in range(ngroups):
                lo, hi = j * 128, (j + 1) * 128
                for (b, s0, s1) in bounds:
                    if s0 < hi and s1 > lo:
                        mms.append((j, b))
            byj = {}
            for (j, b) in mms:
                byj.setdefault(j, []).append(b)
            mms_first, mms_cross = [], []
            for j in sorted(byj):
                bs = byj[j]
                assert len(bs) <= 2, bs
                mms_first.append((j, bs[0]))
                if len(bs) > 1:
                    mms_cross.append((j, bs[1]))
            sections.append({"pass": p, "w": w, "ngroups": ngroups,
                             "mms_first": mms_first,
                             "mms_cross": mms_cross,
                             "start_slot": sec_start})
    S = slot_base
    nseg = sid
    seg_sizes = np.array(seg_sizes, np.int64)
    seg_win = np.array(seg_win, np.int64)
    seg_offs = np.zeros(nseg + 1, np.int64)
    np.cumsum(seg_sizes, out=seg_offs[1:])
    win_of_slot = np.repeat(seg_win, seg_sizes)
    zrow = np.array([PSZ[w] for w in range(NP_)], np.int64)

    # per-group base code (first overlapping block-in-pass * 128)
    gbase = np.zeros(S // BLK, np.float32)

    bip = np.zeros(NBLK, np.int64)
    for p in range(NP_):
        for i, b in enumerate(PASS_BLOCKS[p]):
            bip[b] = i
    for sec in sections:
        g0 = sec["start_slot"] // BLK
        for (j, b) in sec["mms_first"]:
            gbase[g0 + j] = bip[b] * BLK
    base_of_slot = np.repeat(gbase, BLK)

    per_core = []
    for c in range(NCORES):
        m = core == c
        key = seg_id_of[blk[m], piece[m]]
        order = np.argsort(key, kind="stable")
        ks = key[order]
        bstart = np.searchsorted(ks, np.arange(nseg))
        rank = np.arange(len(ks)) - bstart[ks]
        slot = seg_offs[ks] + rank

        so_cor = scor[m][order]
        so_loc = sloc[m][order]
        so_p = piece[m][order]
        row = so_cor * np.array(PSZE)[so_p] + (so_loc - np.array(PSTART)[so_p])

        gidx = zrow[win_of_slot].astype(np.int16)   # pad -> window zero row
        gidx[slot] = row.astype(np.int16)
        code = np.full(S, -1.0, np.float32)
        code[slot] = (bip[blk[m][order]] * BLK
                      + dloc[m][order] % BLK).astype(np.float32)
        # shifted codes: cs1 selects the group's first block (values in
        # [0,128) there), cs2 the second; everything else falls outside
        # [0,128) so the chunk-wide iota is_equal gives all-zero rows.
        cs1 = code - base_of_slot
        cs2 = cs1 - BLK

        gidx16 = gidx.reshape(S // 16, 16).T      # [16, S/16]
        gidx_rep = np.tile(gidx16, (8, 1)).copy() # replicated for Q7 cores
        # per slot, [128, S/128]: partition = slot % 128, col = group
        cs1G = np.ascontiguousarray(cs1.reshape(S // BLK, BLK).T)
        cs2G = np.ascontiguousarray(cs2.reshape(S // BLK, BLK).T)

        degc = np.ones(NBLK * BLK, np.float32)
        degc[:NPC] = deg[c * NPC:(c + 1) * NPC]
        degT = degc.reshape(NBLK, BLK).T.copy()   # [128, NBLK]

        per_core.append({"gidx": gidx_rep, "cs1G": cs1G, "cs2G": cs2G,
                         "degT": degT})

    return sections, S, per_core


def _chunks_of(n):
    """Split n groups into near-equal chunks of at most CHUNK_GROUPS."""
    k = -(-n // CHUNK_GROUPS)
    base, rem = divmod(n, k)
    out = []
    s = 0
    for i in range(k):
        sz = base + (1 if i < rem else 0)
        out.append((s, sz))
        s += sz
    return out


def _build(sections, S, b1_nonzero, b2_nonzero):
    import concourse.bacc as bacc
    import concourse.mybir as mybir
    import concourse.tile as tile
    from concourse.masks import make_identity
    from bass_rust import add_dep_helper

    f32 = mybir.dt.float32
    bf16 = mybir.dt.bfloat16
    AT = mybir.AluOpType

    bip = {}
    pos = {}
    for p in range(NP_):
        for i, b in enumerate(PASS_BLOCKS[p]):
            bip[b] = i
            pos[b] = divmod(i, 8)

    # first/last matmul (global emission index) per (pass, bank);
    # emission order: per chunk, first-block matmuls then crossings.
    first, last = {}, {}
    mmidx = 0
    for sec in sections:
        p = sec["pass"]
        for (cs, cn) in _chunks_of(sec["ngroups"]):
            for mlist in (sec["mms_first"], sec["mms_cross"]):
                for (j, b) in mlist:
                    if cs <= j < cs + cn:
                        bank, _ = pos[b]
                        last[(p, bank)] = mmidx
                        first.setdefault((p, bank), mmidx)
                        mmidx += 1

    nc = bacc.Bacc("TRN2", target_bir_lowering=False, debug=False,
                   enable_asserts=False, num_devices=NCORES,
                   num_swdge_queues=4)
    xT = nc.dram_tensor("xT", [F_IN, NPC], bf16, kind="ExternalInput")
    W1 = nc.dram_tensor("W1", [F_IN, F_HID], bf16, kind="ExternalInput")
    W2 = nc.dram_tensor("W2", [F_HID, F_OUT], f32, kind="ExternalInput")
    b1r = nc.dram_tensor("b1r", [BLK, F_HID], f32, kind="ExternalInput")
    b2r = nc.dram_tensor("b2r", [BLK, F_OUT], f32, kind="ExternalInput")
    degT = nc.dram_tensor("degT", [BLK, NBLK], f32, kind="ExternalInput")
    gidx = nc.dram_tensor("gidx", [BLK, S // 16], mybir.dt.int16,
                          kind="ExternalInput")
    cs1G = nc.dram_tensor("cs1G", [BLK, S // BLK], f32,
                          kind="ExternalInput")
    cs2G = nc.dram_tensor("cs2G", [BLK, S // BLK], f32,
                          kind="ExternalInput")
    iotab = nc.dram_tensor("iotab", [BLK, BLK], f32, kind="ExternalInput")
    y = nc.dram_tensor("y", [NPC, F_OUT], f32, kind="ExternalOutput")

    with tile.TileContext(nc) as tc:
        with tc.tile_pool(name="const", bufs=1) as const, \
             tc.tile_pool(name="xt", bufs=3) as xpool, \
             tc.tile_pool(name="hs", bufs=3) as hpool, \
             tc.tile_pool(name="msgs", bufs=6) as mpool, \
             tc.tile_pool(name="oh", bufs=3) as ohpool, \
             tc.tile_pool(name="post", bufs=3) as ppool, \
             tc.tile_pool(name="psb", bufs=1, space="PSUM") as psb, \
             tc.tile_pool(name="pst", bufs=2, space="PSUM") as pst, \
             tc.tile_pool(name="dram", bufs=1, space="DRAM") as dram:

            ag_in = [[dram.tile([PSZE[p], TBLW], bf16,
                                name=f"agin{L}_{p}", tag=f"agin{L}_{p}")
                      for p in range(NP_)] for L in range(2)]
            ag_out = [[dram.tile([8 * PSZE[p], TBLW], bf16,
                                 addr_space="Shared",
                                 name=f"agout{L}_{p}", tag=f"agout{L}_{p}")
                       for p in range(NP_)] for L in range(2)]

            # ---- constants ----
            ident = const.tile([BLK, BLK], f32)
            make_identity(nc, ident[:])
            W1t = const.tile([F_IN, F_HID], bf16)
            nc.sync.dma_start(W1t[:], W1[:])
            W2t = const.tile([F_HID, F_OUT], f32)
            nc.sync.dma_start(W2t[:], W2[:])
            if b1_nonzero:
                b1t = const.tile([BLK, F_HID], f32)
                nc.sync.dma_start(b1t[:], b1r[:])
            if b2_nonzero:
                b2t = const.tile([BLK, F_OUT], f32)
                nc.sync.dma_start(b2t[:], b2r[:])
            degt = const.tile([BLK, NBLK], f32)
            nc.sync.dma_start(degt[:], degT[:])
            rcp = const.tile([BLK, NBLK], f32)
            nc.vector.reciprocal(rcp[:], degt[:])
            dinv = const.tile([BLK, NBLK], f32)
            nc.scalar.sqrt(dinv[:], rcp[:])
            dinv2 = const.tile([BLK, NBLK], f32)
            nc.vector.tensor_mul(dinv2[:], dinv[:], dinv[:])
            idx_sb = const.tile([BLK, S // 16], mybir.dt.int16)
            nc.sync.dma_start(idx_sb[:], gidx[:])
            cs1_sb = const.tile([BLK, S // BLK], f32)
            nc.sync.dma_start(cs1_sb[:], cs1G[:])
            cs2_sb = const.tile([BLK, S // BLK], f32)
            nc.sync.dma_start(cs2_sb[:], cs2G[:])
            iota_sb = const.tile([BLK, BLK], f32)
            nc.sync.dma_start(iota_sb[:], iotab[:])
            zt = const.tile([2, TBLW], bf16)
            nc.gpsimd.memset(zt[:], 0.0)
            for L in range(2):
                for p in range(NP_):
                    nc.sync.dma_start(
                        ag_in[L][p][PSZ[p]:PSZ[p] + 2, :], zt[:])

            # SBUF-resident fp32 copies of the local pre-scaled rows for
            # the elementwise self-loop term (dinv^2 * h == dinv * hs).
            hs1_all = const.tile([BLK, NBLK * F_HID], f32)
            hs2_all = const.tile([BLK, NBLK * F_HID], f32)
            nc.gpsimd.memset(hs1_all[:], 0.0)
            nc.gpsimd.memset(hs2_all[:], 0.0)

            def piece_of_block(b):
                return min(b // 25, NP_ - 1)

            # ---- phase 1: h_scaled = dinv * (x @ W1), locally owned ----
            for p in range(NP_):
                for t in PASS_BLOCKS[p]:
                    nt = BLK if t < NBLK - 1 else LAST_BLK
                    xt = xpool.tile([F_IN, BLK], bf16, tag="xt")
                    nc.sync.dma_start(xt[:, :nt], xT[:, t * BLK:t * BLK + nt])
                    hp = pst.tile([BLK, 512], f32, space="PSUM", tag="tmp",
                                  name="hp")
                    nc.tensor.matmul(out=hp[:nt, :F_HID], lhsT=xt[:, :nt],
                                     rhs=W1t[:], start=True, stop=True)
                    nc.vector.tensor_scalar(
                        out=hs1_all[:nt, t * F_HID:(t + 1) * F_HID],
                        in0=hp[:nt, :F_HID],
                        scalar1=dinv[:nt, t:t + 1], scalar2=None,
                        op0=AT.mult)
                    hsb = hpool.tile([BLK, TBLW], bf16, tag="hs")
                    nc.scalar.activation(
                        hsb[:nt, :F_HID],
                        hs1_all[:nt, t * F_HID:(t + 1) * F_HID],
                        func=mybir.ActivationFunctionType.Copy)
                    r0 = t * BLK - PSTART[p]
                    nc.sync.dma_start(ag_in[0][p][r0:r0 + nt, :], hsb[:nt, :])
                nc.gpsimd.collective_compute(
                    "AllGather", AT.bypass,
                    replica_groups=[list(range(NCORES))],
                    ins=[ag_in[0][p].opt()],
                    outs=[ag_out[0][p].opt()],
                )

            ag2_insts = {}
            anchor = {}       # (L, pass, w, chunk) -> first gather inst

            def run_layer(L, post_fn, after_pass=None):
                tables = ag_out[L]
                mmcount = [0]
                banks = {}

                for sec in sections:
                    p, w = sec["pass"], sec["w"]
                    blocks = PASS_BLOCKS[p]
                    if w == 0:
                        banks.clear()
                        for b in blocks:
                            bank, _ = pos[b]
                            if bank not in banks:
                                banks[bank] = psb.tile(
                                    [BLK, 512], f32, space="PSUM",
                                    name=f"bank{bank}", tag=f"bank{bank}")
                    g0_global = sec["start_slot"] // BLK
                    chunks = _chunks_of(sec["ngroups"])
                    for ci, (cs, cn) in enumerate(chunks):
                        mt = mpool.tile([BLK, CHUNK_GROUPS, TBLW], bf16,
                                        tag="msgs")
                        nsub = min(4, cn)
                        base, rem = divmod(cn, nsub)
                        j0 = 0
                        for si in range(nsub):
                            sg = base + (1 if si < rem else 0)
                            if sg == 0:
                                continue
                            sn = sg * BLK
                            soff = g0_global + cs + j0
                            gi = nc.gpsimd.dma_gather(
                                out_ap=mt[:, j0:j0 + sg, :],
                                in_ap=tables[w][:, :],
                                idxs_ap=idx_sb[:, soff * 8:
                                               soff * 8 + sn // 16],
                                num_idxs=sn, num_idxs_reg=sn,
                                elem_size=TBLW,
                                single_packet=False,
                                queue_num=si,
                            )
                            if si == 0:
                                anchor[(L, p, w, ci)] = gi
                            j0 += sg
                        firsts = [(j, b) for (j, b) in sec["mms_first"]
                                  if cs <= j < cs + cn]
                        crosses = [(j, b) for (j, b) in sec["mms_cross"]
                                   if cs <= j < cs + cn]
                        ib = iota_sb[:, :].unsqueeze(1)
                        oh1 = ohpool.tile([BLK, CHUNK_GROUPS, BLK], bf16,
                                          tag="oh1")
                        nc.vector.tensor_tensor(
                            out=oh1[:, :cn, :],
                            in0=ib.to_broadcast([BLK, cn, BLK]),
                            in1=cs1_sb[:, g0_global + cs:
                                       g0_global + cs + cn]
                                .to_broadcast([BLK, cn, BLK]),
                            op=AT.is_equal)
                        if crosses:
                            oh2 = ohpool.tile([BLK, CHUNK_GROUPS, BLK],
                                              bf16, tag="oh2")
                            nc.vector.tensor_tensor(
                                out=oh2[:, :cn, :],
                                in0=ib.to_broadcast([BLK, cn, BLK]),
                                in1=cs2_sb[:, g0_global + cs:
                                           g0_global + cs + cn]
                                    .to_broadcast([BLK, cn, BLK]),
                                op=AT.is_equal)
                        pairs = [(oh1, firsts)]
                        if crosses:
                            pairs.append((oh2, crosses))
                        for oh, mlist in pairs:
                            for (j, b) in mlist:
                                bank, off = pos[b]
                                gm = mmcount[0]
                                mmcount[0] += 1
                                nc.tensor.matmul(
                                    out=banks[bank][:, off * F_HID:
                                                    (off + 1) * F_HID],
                                    lhsT=oh[:, j - cs, :],
                                    rhs=mt[:, j - cs, :F_HID],
                                    start=(gm == first[(p, bank)]),
                                    stop=(gm == last[(p, bank)]),
                                    skip_group_check=True)
                    if w == NP_ - 1:
                        for bank, bt in banks.items():
                            bank_blocks = [b for b in blocks
                                           if pos[b][0] == bank]
                            post_fn(bank, bt, bank_blocks)
                        if after_pass is not None:
                            after_pass(p)

            # ---- layer 1 post:
            # X = bank + dinv*hs1 (self-loop); h1s = dinv*relu(dinv*X + b1)
            # b1 == 0 fast path: dinv*relu(dinv*X) == dinv^2*relu(X).
            def post1(bank, bt, bank_blocks):
                for i, b in enumerate(bank_blocks):
                    nb = BLK if b < NBLK - 1 else LAST_BLK
                    sl = bt[:, i * F_HID:(i + 1) * F_HID]
                    hb = hs1_all[:, b * F_HID:(b + 1) * F_HID]
                    x = ppool.tile([BLK, F_HID], f32, tag="post1x", name="x")
                    nc.vector.tensor_add(out=x[:], in0=hb, in1=sl)
                    sl2 = hs2_all[:, b * F_HID:(b + 1) * F_HID]
                    if b1_nonzero:
                        h = ppool.tile([BLK, F_HID], f32, tag="post1",
                                       name="h")
                        nc.vector.tensor_scalar(out=h[:], in0=x[:],
                                                scalar1=dinv[:, b:b + 1],
                                                scalar2=None, op0=AT.mult)
                        nc.vector.tensor_add(out=h[:], in0=h[:], in1=b1t[:])
                        nc.vector.tensor_scalar(out=sl2, in0=h[:],
                                                scalar1=dinv[:, b:b + 1],
                                                scalar2=0.0, op0=AT.mult,
                                                op1=AT.max)
                    else:
                        nc.scalar.activation(
                            x[:], x[:],
                            func=mybir.ActivationFunctionType.Relu)
                        nc.vector.tensor_scalar(out=sl2, in0=x[:],
                                                scalar1=dinv2[:, b:b + 1],
                                                scalar2=None, op0=AT.mult)
                    hbf = ppool.tile([BLK, TBLW], bf16, tag="post1b",
                                     name="hbf")
                    nc.scalar.activation(
                        hbf[:, :F_HID], sl2,
                        func=mybir.ActivationFunctionType.Copy)
                    pp = piece_of_block(b)
                    r0 = b * BLK - PSTART[pp]
                    nc.sync.dma_start(ag_in[1][pp][r0:r0 + nb, :],
                                      hbf[:nb, :])

            def ag2_piece(p):
                cc = nc.gpsimd.collective_compute(
                    "AllGather", AT.bypass,
                    replica_groups=[list(range(NCORES))],
                    ins=[ag_in[1][p].opt()],
                    outs=[ag_out[1][p].opt()],
                )
                ag2_insts[p] = cc

            run_layer(0, post1, after_pass=ag2_piece)

            # ---- layer 2 post: out = dinv*((bank + dinv*hs2) @ W2) + b2 --
            def post2(bank, bt, bank_blocks):
                for i, b in enumerate(bank_blocks):
                    nb = BLK if b < NBLK - 1 else LAST_BLK
                    sl = bt[:, i * F_HID:(i + 1) * F_HID]
                    hb = hs2_all[:, b * F_HID:(b + 1) * F_HID]
                    ag = ppool.tile([BLK, F_HID], f32, tag="agg2", name="ag")
                    nc.vector.tensor_add(out=ag[:], in0=hb, in1=sl)
                    t2 = pst.tile([BLK, 512], f32, space="PSUM", tag="tmp",
                                  name="t2")
                    nc.tensor.transpose(
                        out=t2[0:F_HID, 0:BLK],
                        in_=ag[:],
                        identity=ident[:])
                    aT = ppool.tile([F_HID, BLK], f32, tag="aggT", name="aT")
                    nc.scalar.activation(aT[:], t2[0:F_HID, 0:BLK],
                                         func=mybir.ActivationFunctionType.Copy)
                    nc.tensor.matmul(out=t2[:, BLK:BLK + F_OUT], lhsT=aT[:],
                                     rhs=W2t[:], start=True, stop=True)
                    o = ppool.tile([BLK, F_OUT], f32, tag="out2", name="o")
                    nc.vector.tensor_scalar(out=o[:],
                                            in0=t2[:, BLK:BLK + F_OUT],
                                            scalar1=dinv[:, b:b + 1],
                                            scalar2=None, op0=AT.mult)
                    if b2_nonzero:
                        nc.vector.tensor_add(out=o[:], in0=o[:], in1=b2t[:])
                    nc.sync.dma_start(y[b * BLK:b * BLK + nb, :], o[:nb, :])

            run_layer(1, post2)

            # Pin each AG2 trigger early in the Pool stream: a gather a
            # couple chunks into the NEXT pass (or layer-2 start for the
            # last pass) must come after it, so the scheduler cannot
            # defer the trigger to layer-2's first use of the table.
            for p in range(NP_):
                if p < NP_ - 1:
                    frm = anchor.get((0, p + 1, 0, 1)) or \
                          anchor.get((0, p + 1, 0, 0))
                else:
                    frm = anchor.get((1, 0, 0, 1)) or anchor.get((1, 0, 0, 0))
                if frm is not None and p in ag2_insts:
                    add_dep_helper(frm.ins, ag2_insts[p].ins, True,
                                   f"pin AG2_{p} trigger early")

    nc.compile()
    return nc


def _to_bf16(a):
    import ml_dtypes
    return np.asarray(a, dtype=np.float32).astype(ml_dtypes.bfloat16)


def kernel(x, W1, b1, W2, b2, edge_index):
    global LAST_RESULTS
    from concourse.bass_utils import run_bass_kernel_spmd

    x = np.asarray(x, dtype=np.float32)
    W1 = np.asarray(W1, dtype=np.float32)
    W2 = np.asarray(W2, dtype=np.float32)
    b1 = np.asarray(b1, dtype=np.float32)
    b2 = np.asarray(b2, dtype=np.float32)

    ekey = hash(np.asarray(edge_index).tobytes()) ^ hash(
        (bool(np.any(b1)), bool(np.any(b2))))
    if ekey in _CACHE:
        nc, sections, S, per_core = _CACHE[ekey]
    else:
        sections, S, per_core = _preprocess(edge_index)
        nc = _build(sections, S, bool(np.any(b1)), bool(np.any(b2)))
        _CACHE.clear()
        _CACHE[ekey] = (nc, sections, S, per_core)

    b1r = np.broadcast_to(b1, (BLK, F_HID)).copy()
    b2r = np.broadcast_to(b2, (BLK, F_OUT)).copy()
    W1b = _to_bf16(W1)
    iotab = np.tile(np.arange(BLK, dtype=np.float32), (BLK, 1))
    in_maps = []
    for c in range(NCORES):
        pc = per_core[c]
        in_maps.append({
            "xT": _to_bf16(np.ascontiguousarray(x[c * NPC:(c + 1) * NPC].T)),
            "W1": W1b, "W2": W2, "b1r": b1r, "b2r": b2r,
            "degT": pc["degT"], "gidx": pc["gidx"],
            "cs1G": pc["cs1G"], "cs2G": pc["cs2G"], "iotab": iotab,
        })

    res = run_bass_kernel_spmd(nc, in_maps, core_ids=list(range(NCORES)))
    LAST_RESULTS = res
    return np.concatenate([res.results[c]["y"] for c in range(NCORES)], axis=0)
